# revision 22
# baseline (speedup 1.0000x reference)
"""Causal single-head attention on 8 Trainium2 NeuronCores.

Problem: x [8, 2048, 1024] f32, Wq/Wk/Wv [1024, 64] f32.
  q = x@Wq, k = x@Wk, v = x@Wv
  att = softmax(mask(q k^T / sqrt(1024)))
  out = att @ v          -> [8, 2048, 64] f32

Sharding: data-parallel over batch, one batch element per core; weights
replicated. Per-core kernel layout choices (v4):

 * All matmul operands are bf16 (accumulation stays f32 in PSUM). x is
   cast to bf16 on host and pre-transposed to xT [E, S] slab layout,
   halving HBM traffic vs f32. End-to-end error ~3e-3, inside the 2e-2
   gate (scores are ~N(0, 0.083^2) so exp never overflows).
 * Constants (identity f32, identity bf16 | triu bf16) are DMA'd from
   DRAM instead of built with GpSimd affine_selects — the on-chip build
   serialized ~7us of preamble before the first matmul.
 * Projections run as two chains over the same x stream:
   chain A stationary [Wq | Wv] -> PSUM rows 0-63 = Q^T, 64-127 = V^T;
   chain B stationary [Wk]      -> PSUM rows 0-63 = K^T.
   V^T is transposed to natural [k, h] layout with an identity block at
   partitions 64-127 (ident[64:128, 64:128] of a 128x128 identity).
 * ALL attention matmuls use full [128, 128] stationaries: K^T is padded
   with 64 zero rows so the score matmul can take the whole qvT tile
   (Q^T rows 0-63, V^T rows 64-127) as its moving operand — the V^T rows
   multiply the zero rows of the stationary and contribute nothing. V' is
   padded with zero columns 65-127. Uniform full-width stationaries keep
   the fast-weight-load path on and measured ~100ns/matmul faster than
   the narrow [64,128]/[128,65] versions.
 * Scores are computed TRANSPOSED (att^T[k, q] blocks, K^T-stationary) so
   the exp'd P^T blocks feed the PV matmul directly as the moving operand.
   Score matmuls run DEPTH=4 blocks ahead of the PV matmuls in PE program
   order so the PE never stalls on the ScalarE exp (the stall otherwise
   also re-throttles the HAM clock gate).
 * Softmax uses no max-subtraction; masked entries are exp'd then zeroed
   by a 0/1 triangular mask (diagonal blocks only; strictly-upper blocks
   are never computed).
 * V' carries a ones-column (col 64): the PV matmul yields the softmax
   denominator l as out^T row 64 for free. A final PE transpose per 128-q
   block brings out back to natural layout where the divide is a
   per-partition tensor_scalar op; slab J's output stage is deferred
   behind slab J+1's projection chains to keep the PE dense.
 * _legalize_waits post-processes the scheduled BIR: the TPB ISA encodes a
   single sem-wait per instruction and several walrus lowerings reject
   more, so excess waits move onto injected same-engine NoOps.
"""

import numpy as np

B, S, E, H = 8, 2048, 1024, 64
SC = 512            # s/q-chunk width (PSUM bank = 512 f32)
NSC = S // SC       # 4 chunks
NQB = S // 128      # 16 q/k blocks
NET = E // 128      # 8 e-tiles
SCALE = float(E) ** -0.5
NWARM = 34          # PE clock-gate warm-up fillers bridging the input DMA
DEPTH = 2           # score-pair units run this many units ahead of the PVs

_CACHE = {}


def _build_bass():
    import concourse.bass as bass
    import concourse.tile as tile
    from concourse import mybir

    f32 = mybir.dt.float32
    bf16 = mybir.dt.bfloat16
    Exp = mybir.ActivationFunctionType.Exp

    nc = bass.Bass()
    # xs[sc, p, t, s] = x[b].T[t*128+p, sc*512+s]: each (slab, e-piece) DMA
    # reads fully contiguous per-partition lines.
    xs = nc.dram_tensor("xs", [NSC, 128, NET, SC], bf16, kind="ExternalInput")
    wa = nc.dram_tensor("wa", [E, 128], bf16, kind="ExternalInput")  # [Wq|Wv]
    wb = nc.dram_tensor("wb", [E, H], bf16, kind="ExternalInput")    # Wk
    # cb = [identity | upper-triangular(k<=q)] bf16; cf = identity f32
    cb = nc.dram_tensor("cb", [128, 256], bf16, kind="ExternalInput")
    cf = nc.dram_tensor("cf", [128, 128], f32, kind="ExternalInput")
    out = nc.dram_tensor("out", [S, H], f32, kind="ExternalOutput")

    with tile.TileContext(nc) as tc:
        with (
            tc.tile_pool(name="persist", bufs=1) as persist,
            tc.tile_pool(name="work", bufs=4) as work,
            tc.tile_pool(name="ptp", bufs=6) as ptp,
            tc.tile_pool(name="pblk", bufs=5, space="PSUM") as pblk,
            tc.tile_pool(name="pout", bufs=1, space="PSUM") as pout,
            tc.tile_pool(name="psml", bufs=2, space="PSUM") as psml,
        ):
            # --- constants (DMA'd; sync ring first so warm-up starts fast) -
            cst_bf = persist.tile([128, 256], bf16)  # [:,0:128] I, [:,128:256] triu
            cst_f = persist.tile([128, 128], f32)
            nc.sync.dma_start(out=cst_bf[:], in_=cb[:])
            ident = cst_f
            ident_bf = cst_bf[:, 0:128]
            triu = cst_bf[:, 128:256]

            zbias = persist.tile([128, 1], f32)
            nc.vector.memset(zbias[:], 0.0)
            # V' stationary [k, 128]: cols 0-63 V (written per slab), col 64
            # ones (denominator trick), cols 65-127 zero padding.
            vp_sb = persist.tile([128, NQB, 128], bf16)
            nc.vector.memset(vp_sb[:, :, H:128], 0.0)
            nc.vector.memset(vp_sb[:, :, H : H + 1], 1.0)
            # K^T stationary rows 64-127 are zero padding (so the score
            # matmul's moving operand can be the whole qvT tile).
            kT_pad = persist.tile([128, S], bf16)
            nc.vector.memset(kT_pad[64:128, :], 0.0)

            # --- PE warm-up -----------------------------------------------
            # The PE clock gate starts at 1.2 GHz and reaches 2.4 GHz after
            # ~3.4us of sustained matmul activity. Burn fillers into a
            # rotating PSUM buf while the input DMAs stream. A dummy exp on
            # ScalarE pulls the ~2.7us ACT_TABLE_LOAD off the critical path.
            warm_ps = pblk.tile([128, SC], f32, tag="blk")
            for _ in range(NWARM):
                nc.tensor.matmul(
                    warm_ps[:, 0:128], lhsT=triu, rhs=triu,
                    start=True, stop=True,
                )
            warm_act = persist.tile([128, 1], f32)
            nc.scalar.activation(
                out=warm_act[:], in_=zbias[:], func=Exp, bias=zbias[:], scale=1.0
            )

            # --- load: weights (parallel rings), then x column slabs; the
            # lead-in is HBM-bandwidth-bound (~4us for weights + slab 0), so
            # keep DMA pieces coarse and strictly ring-ordered (a third SWDGE
            # channel would steal HBM bandwidth from the slab-0 stream).
            wa_sb = persist.tile([128, NET, 128], bf16)
            wb_sb = persist.tile([128, NET, H], bf16)
            nc.scalar.dma_start(
                out=wa_sb[:], in_=wa[:].rearrange("(t p) m -> p t m", p=128)
            )
            nc.sync.dma_start(
                out=wb_sb[:], in_=wb[:].rearrange("(t p) m -> p t m", p=128)
            )
            nc.sync.dma_start(out=cst_f[:], in_=cf[:])
            xT_sb = persist.tile([128, NET, S], bf16)

            def xdma(eng, sc, t0, t1):
                eng.dma_start(
                    out=xT_sb[:, t0:t1, sc * SC : (sc + 1) * SC],
                    in_=xs[sc, :, t0:t1, :],
                )

            for t in range(0, NET, 2):
                xdma(nc.scalar if (t // 2) % 2 == 0 else nc.sync, 0, t, t + 2)
            HT = NET // 2
            xdma(nc.scalar, 1, 0, HT)
            xdma(nc.sync, 1, HT, NET)
            for sc in (2, 3):
                xdma(nc.scalar, sc, 0, HT)
                xdma(nc.sync, sc, HT, NET)

            qvT_bf = persist.tile([128, S], bf16)  # rows 0-63 Q^T, 64-127 V^T

            def chains(sc):
                # A and B interleave per e-tile so both finish at DMA pace
                # during the slab-0 lead-in (B otherwise serializes ~1.7us
                # behind A while its data is already on-chip).
                ps = pblk.tile([128, SC], f32, tag="blk")
                psk = pblk.tile([64, SC], f32, tag="blk")
                for t in range(NET):
                    nc.tensor.matmul(
                        ps[:],
                        lhsT=wa_sb[:, t, :],
                        rhs=xT_sb[:, t, sc * SC : (sc + 1) * SC],
                        start=(t == 0),
                        stop=(t == NET - 1),
                    )
                    nc.tensor.matmul(
                        psk[:],
                        lhsT=wb_sb[:, t, :],
                        rhs=xT_sb[:, t, sc * SC : (sc + 1) * SC],
                        start=(t == 0),
                        stop=(t == NET - 1),
                    )
                nc.vector.tensor_copy(qvT_bf[:, sc * SC : (sc + 1) * SC], ps[:])
                nc.vector.tensor_copy(
                    kT_pad[0:64, sc * SC : (sc + 1) * SC], psk[:]
                )

            def vtr(sc):
                # V natural layout for this slab's 4 blocks: one PSUM tile,
                # one batched copy into vp_sb.
                vtp = psml.tile([128, 4, H], bf16, tag="tr")
                for i in range(4):
                    qb = 4 * sc + i
                    nc.tensor.transpose(
                        vtp[:, i, :],
                        qvT_bf[64:128, qb * 128 : (qb + 1) * 128],
                        ident_bf[64:128, 64:128],
                    )
                nc.vector.tensor_copy(vp_sb[:, 4 * sc : 4 * sc + 4, 0:H], vtp[:])

            ops_by_J = {}

            def attention_blocks(J):
                """Scores run DEPTH blocks ahead of PVs in PE program order,
                so the PE never stalls waiting for the ScalarE exp. The last
                DEPTH PVs are NOT emitted here — the returned flush closure
                emits them, and the caller interleaves the next slab's
                projection chain in between to cover their exp latency."""
                ops = pout.tile([128, SC], f32)  # rows 0-63 out^T, 64 l^T
                ops_by_J[J] = ops
                njt = 4 * J + 4                  # k-tiles 0..4J+3
                pts = {}

                def score(j):
                    r = j - 4 * J                # >=0 on diagonal tiles
                    col0 = max(0, r * 128)
                    # att^T block [k=128, q]: zero-padded K^T stationary x
                    # full qvT moving (V^T rows hit the zero rows)
                    aps = pblk.tile([128, SC], f32, tag="blk")
                    nc.tensor.matmul(
                        aps[:, col0:SC],
                        lhsT=kT_pad[:, j * 128 : (j + 1) * 128],
                        rhs=qvT_bf[:, J * SC + col0 : (J + 1) * SC],
                        start=True,
                        stop=True,
                    )
                    # P^T = exp(att^T / sqrt(E)); zero the k>q corner on the
                    # diagonal block
                    pt = ptp.tile([128, SC], bf16, tag="pt")
                    nc.scalar.activation(
                        out=pt[:, col0:SC],
                        in_=aps[:, col0:SC],
                        func=Exp,
                        bias=zbias[:],
                        scale=SCALE,
                    )
                    if r >= 0:
                        nc.vector.tensor_mul(
                            pt[:, col0 : col0 + 128],
                            pt[:, col0 : col0 + 128],
                            triu[:],
                        )
                    pts[j] = (pt, col0)

                def pv(j):
                    pt, col0 = pts.pop(j)
                    # out^T[:, col0:] += V'_j^T @ P^T_j
                    nc.tensor.matmul(
                        ops[:, col0:SC],
                        lhsT=vp_sb[:, j, :],
                        rhs=pt[:, col0:SC],
                        start=(j == 0),
                        stop=(j == njt - 1),
                    )

                for j in range(njt):
                    score(j)
                    if j >= DEPTH:
                        pv(j - DEPTH)

                def flush():
                    for j in range(max(0, njt - DEPTH), njt):
                        pv(j)

                return flush

            def attention_out(J):
                # transpose back per 128-q block, divide by l, batched store
                ops = ops_by_J.pop(J)
                osb = work.tile([H + 1, SC], f32, tag="osb")
                nc.vector.tensor_copy(osb[:], ops[0 : H + 1, :])
                obs = work.tile([128, 4, H], f32, tag="ob")
                for c in range(4):
                    tp2 = psml.tile([128, H + 1], f32, tag="tr")
                    nc.tensor.transpose(
                        tp2[:],
                        osb[:, c * 128 : (c + 1) * 128],
                        ident[0 : H + 1, 0 : H + 1],
                    )
                    rcp = work.tile([128, 1], f32, tag="rcp")
                    nc.vector.reciprocal(rcp[:], tp2[:, H : H + 1])
                    nc.vector.tensor_scalar_mul(
                        obs[:, c, :], in0=tp2[:, 0:H], scalar1=rcp[:]
                    )
                # one SWDGE store per slab; keeps stores off the input rings
                nc.gpsimd.dma_start(
                    out=out[J * SC : (J + 1) * SC, :].rearrange(
                        "(c p) h -> p c h", p=128
                    ),
                    in_=obs[:],
                )

            # Schedule: the tail PVs of slab J flush BETWEEN the two
            # projection chains of slab J+1, so their exps complete under
            # chain-A matmuls; slab J's output stage runs after slab J+1's
            # projections so the PE never waits on the DVE osb copy.
            chains(0)
            vtr(0)
            flush = attention_blocks(0)
            for sc in range(1, NSC):
                chains(sc)
                flush()
                vtr(sc)
                attention_out(sc - 1)
                flush = attention_blocks(sc)
            flush()
            attention_out(NSC - 1)
    return nc


def _legalize_waits(nc):
    """Split multi-wait instructions: the TPB ISA encodes one sem-wait per
    instruction and several walrus struct lowerings (Activation, self-loading
    Matmult, DMA direct2d, NoOp/Drain) reject more ("Too many sync wait
    commands"). Move excess waits onto inserted same-engine NoOps, one wait
    each. EventSemaphore handles wait lists natively - leave it."""
    from concourse import mybir

    skip = (mybir.InstEventSemaphore,)
    n = 0
    for f in nc.m.functions:
        for bb in f.blocks:
            new = []
            for inst in bb.instructions:
                si = inst.sync_info
                waits = list(si.on_wait) if si is not None else []
                if len(waits) > 1 and not isinstance(inst, skip):
                    for w in waits[:-1]:
                        n += 1
                        nop = mybir.InstNoOp(
                            name=f"I-waitsplit-{n}", ins=[], outs=[]
                        )
                        nop.engine = inst.engine
                        nop.sync_info = mybir.SyncInfo(on_wait=[w], on_update=[])
                        new.append(nop)
                    inst.sync_info = mybir.SyncInfo(
                        on_wait=[waits[-1]], on_update=list(si.on_update)
                    )
                new.append(inst)
            bb.instructions[:] = new
    return n


def _get_nc():
    if "nc" not in _CACHE:
        nc = _build_bass()
        _legalize_waits(nc)
        _CACHE["nc"] = nc
    return _CACHE["nc"]


def _bf16():
    import ml_dtypes

    return ml_dtypes.bfloat16


def _prep_x(xb):
    """[S, E] batch element -> xs[sc, p, t, s] bf16 slab-contiguous layout."""
    return np.ascontiguousarray(
        xb.T.astype(_bf16()).reshape(NET, 128, NSC, SC).transpose(2, 1, 0, 3)
    )


def _consts():
    bf16 = _bf16()
    ident = np.eye(128, dtype=np.float32)
    triu = np.triu(np.ones((128, 128), dtype=np.float32))  # 1 where k <= q
    cb = np.ascontiguousarray(
        np.concatenate([ident, triu], axis=1).astype(bf16)
    )
    cf = np.ascontiguousarray(ident)
    return cb, cf


def _in_maps(x, Wq, Wk, Wv):
    bf16 = _bf16()
    x = np.asarray(x, dtype=np.float32)
    wa = np.ascontiguousarray(
        np.concatenate(
            [np.asarray(Wq, np.float32), np.asarray(Wv, np.float32)], axis=1
        ).astype(bf16)
    )
    wb = np.ascontiguousarray(np.asarray(Wk, np.float32).astype(bf16))
    cb, cf = _consts()
    return [
        {"xs": _prep_x(x[b]), "wa": wa, "wb": wb, "cb": cb, "cf": cf}
        for b in range(B)
    ]


def kernel(x, Wq, Wk, Wv):
    from concourse.bass_utils import run_bass_kernel_spmd

    nc = _get_nc()
    in_maps = _in_maps(x, Wq, Wk, Wv)
    res = run_bass_kernel_spmd(nc, in_maps, core_ids=list(range(B)))
    return np.stack([res.results[b]["out"] for b in range(B)], axis=0)


# revision 24
# speedup vs baseline: 1.0285x; 1.0285x over previous
"""Causal single-head attention on 8 Trainium2 NeuronCores.

Problem: x [8, 2048, 1024] f32, Wq/Wk/Wv [1024, 64] f32.
  q = x@Wq, k = x@Wk, v = x@Wv
  att = softmax(mask(q k^T / sqrt(1024)))
  out = att @ v          -> [8, 2048, 64] f32

Sharding: data-parallel over batch, one batch element per core; weights
replicated. Per-core kernel layout choices:

 * All matmul operands are bf16 (accumulation stays f32 in PSUM). x is
   cast to bf16 on host and pre-transposed to xT [E, S] slab layout,
   halving HBM traffic vs f32. End-to-end error ~3e-3, inside the 2e-2
   gate (scores are ~N(0, 0.083^2) so exp never overflows).
 * Constants (identity f32, identity bf16 | triu bf16) are DMA'd from
   DRAM instead of built with GpSimd affine_selects — the on-chip build
   serialized ~7us of preamble before the first matmul.
 * Projections run as two chains over the same x stream:
   chain A stationary [Wq | Wv] -> PSUM rows 0-63 = Q^T, 64-127 = V^T;
   chain B stationary [Wk]      -> PSUM rows 0-63 = K^T.
   V^T is transposed to natural [k, h] layout with an identity block at
   partitions 64-127 (ident[64:128, 64:128] of a 128x128 identity).
 * ALL attention matmuls use full [128, 128] stationaries: K^T is padded
   with 64 zero rows so the score matmul can take the whole qvT tile
   (Q^T rows 0-63, V^T rows 64-127) as its moving operand — the V^T rows
   multiply the zero rows of the stationary and contribute nothing. V' is
   padded with zero columns 65-127. Uniform full-width stationaries keep
   the fast-weight-load path on and measured ~100ns/matmul faster than
   the narrow [64,128]/[128,65] versions.
 * Scores are computed TRANSPOSED (att^T[k, q] blocks, K^T-stationary) so
   the exp'd P^T blocks feed the PV matmul directly as the moving operand.
   Score matmuls run DEPTH blocks ahead of the PV matmuls in PE program
   order so the PE never stalls on the ScalarE exp (the stall otherwise
   also re-throttles the HAM clock gate); each slab's last DEPTH PVs are
   deferred behind the next slab's chain-A matmuls for the same reason.
 * Softmax uses no max-subtraction; masked entries are exp'd then zeroed
   by a 0/1 triangular mask (diagonal blocks only; strictly-upper blocks
   are never computed).
 * V' carries a ones-column (col 64): the PV matmul yields the softmax
   denominator l as out^T row 64 for free. A final PE transpose per 128-q
   block brings out back to natural layout where the divide is a
   per-partition tensor_scalar op; slab J's output stage is deferred
   behind slab J+1's projection chains to keep the PE dense.
 * _legalize_waits post-processes the scheduled BIR: the TPB ISA encodes a
   single sem-wait per instruction and several walrus lowerings reject
   more, so excess waits move onto injected same-engine NoOps.
"""

import numpy as np

B, S, E, H = 8, 2048, 1024, 64
SC = 512            # s/q-chunk width (PSUM bank = 512 f32)
NSC = S // SC       # 4 chunks
NQB = S // 128      # 16 q/k blocks
NET = E // 128      # 8 e-tiles
SCALE = float(E) ** -0.5
NWARM = 34          # PE clock-gate warm-up fillers bridging the input DMA
DEPTH = 2           # score-pair units run this many units ahead of the PVs

_CACHE = {}


def _build_bass():
    import concourse.bass as bass
    import concourse.tile as tile
    from concourse import mybir

    f32 = mybir.dt.float32
    bf16 = mybir.dt.bfloat16
    Exp = mybir.ActivationFunctionType.Exp

    nc = bass.Bass()
    # xs[sc, p, t, s] = x[b].T[t*128+p, sc*512+s]: each (slab, e-piece) DMA
    # reads fully contiguous per-partition lines.
    xs = nc.dram_tensor("xs", [NSC, 128, NET, SC], bf16, kind="ExternalInput")
    wa = nc.dram_tensor("wa", [E, 128], bf16, kind="ExternalInput")  # [Wq|Wv]
    wb = nc.dram_tensor("wb", [E, H], bf16, kind="ExternalInput")    # Wk
    # cb = [identity | upper-triangular(k<=q)] bf16; cf = identity f32
    cb = nc.dram_tensor("cb", [128, 256], bf16, kind="ExternalInput")
    cf = nc.dram_tensor("cf", [128, 128], f32, kind="ExternalInput")
    out = nc.dram_tensor("out", [S, H], f32, kind="ExternalOutput")

    with tile.TileContext(nc) as tc:
        with (
            tc.tile_pool(name="persist", bufs=1) as persist,
            tc.tile_pool(name="work", bufs=4) as work,
            tc.tile_pool(name="ptp", bufs=6) as ptp,
            tc.tile_pool(name="pblk", bufs=5, space="PSUM") as pblk,
            tc.tile_pool(name="pout", bufs=1, space="PSUM") as pout,
            tc.tile_pool(name="psml", bufs=2, space="PSUM") as psml,
        ):
            # --- constants (DMA'd; sync ring first so warm-up starts fast) -
            cst_bf = persist.tile([128, 256], bf16)  # [:,0:128] I, [:,128:256] triu
            cst_f = persist.tile([128, 128], f32)
            nc.sync.dma_start(out=cst_bf[:], in_=cb[:])
            ident = cst_f
            ident_bf = cst_bf[:, 0:128]
            triu = cst_bf[:, 128:256]

            zbias = persist.tile([128, 1], f32)
            nc.vector.memset(zbias[:], 0.0)
            # V' stationary [k, 128]: cols 0-63 V (written per slab), col 64
            # ones (denominator trick), cols 65-127 zero padding.
            vp_sb = persist.tile([128, NQB, 128], bf16)
            nc.vector.memset(vp_sb[:, :, H:128], 0.0)
            nc.vector.memset(vp_sb[:, :, H : H + 1], 1.0)
            # K^T stationary rows 64-127 are zero padding (so the score
            # matmul's moving operand can be the whole qvT tile).
            kT_pad = persist.tile([128, S], bf16)
            nc.vector.memset(kT_pad[64:128, :], 0.0)

            # --- PE warm-up -----------------------------------------------
            # The PE clock gate starts at 1.2 GHz and reaches 2.4 GHz after
            # ~3.4us of sustained matmul activity. Burn fillers into a
            # rotating PSUM buf while the input DMAs stream. A dummy exp on
            # ScalarE pulls the ~2.7us ACT_TABLE_LOAD off the critical path.
            warm_ps = pblk.tile([128, SC], f32, tag="blk")
            for _ in range(NWARM):
                nc.tensor.matmul(
                    warm_ps[:, 0:128], lhsT=triu, rhs=triu,
                    start=True, stop=True,
                )
            warm_act = persist.tile([128, 1], f32)
            nc.scalar.activation(
                out=warm_act[:], in_=zbias[:], func=Exp, bias=zbias[:], scale=1.0
            )

            # --- load: weights (parallel rings), then x column slabs; the
            # lead-in is HBM-bandwidth-bound (~4us for weights + slab 0), so
            # keep DMA pieces coarse and strictly ring-ordered (a third SWDGE
            # channel would steal HBM bandwidth from the slab-0 stream).
            wa_sb = persist.tile([128, NET, 128], bf16)
            wb_sb = persist.tile([128, NET, H], bf16)
            nc.scalar.dma_start(
                out=wa_sb[:], in_=wa[:].rearrange("(t p) m -> p t m", p=128)
            )
            nc.sync.dma_start(
                out=wb_sb[:], in_=wb[:].rearrange("(t p) m -> p t m", p=128)
            )
            nc.sync.dma_start(out=cst_f[:], in_=cf[:])
            xT_sb = persist.tile([128, NET, S], bf16)

            def xdma(eng, sc, t0, t1):
                eng.dma_start(
                    out=xT_sb[:, t0:t1, sc * SC : (sc + 1) * SC],
                    in_=xs[sc, :, t0:t1, :],
                )

            for t in range(0, NET, 2):
                xdma(nc.scalar if (t // 2) % 2 == 0 else nc.sync, 0, t, t + 2)
            HT = NET // 2
            xdma(nc.scalar, 1, 0, HT)
            xdma(nc.sync, 1, HT, NET)
            for sc in (2, 3):
                xdma(nc.scalar, sc, 0, HT)
                xdma(nc.sync, sc, HT, NET)

            qvT_bf = persist.tile([128, S], bf16)  # rows 0-63 Q^T, 64-127 V^T

            def chainA(sc):
                ps = pblk.tile([128, SC], f32, tag="blk")
                for t in range(NET):
                    nc.tensor.matmul(
                        ps[:],
                        lhsT=wa_sb[:, t, :],
                        rhs=xT_sb[:, t, sc * SC : (sc + 1) * SC],
                        start=(t == 0),
                        stop=(t == NET - 1),
                    )
                nc.vector.tensor_copy(qvT_bf[:, sc * SC : (sc + 1) * SC], ps[:])

            def chainB(sc):
                psk = pblk.tile([64, SC], f32, tag="blk")
                for t in range(NET):
                    nc.tensor.matmul(
                        psk[:],
                        lhsT=wb_sb[:, t, :],
                        rhs=xT_sb[:, t, sc * SC : (sc + 1) * SC],
                        start=(t == 0),
                        stop=(t == NET - 1),
                    )
                nc.vector.tensor_copy(
                    kT_pad[0:64, sc * SC : (sc + 1) * SC], psk[:]
                )

            def vtr(sc):
                # V natural layout for this slab's 4 blocks: one PSUM tile,
                # one batched copy into vp_sb.
                vtp = psml.tile([128, 4, H], bf16, tag="tr")
                for i in range(4):
                    qb = 4 * sc + i
                    nc.tensor.transpose(
                        vtp[:, i, :],
                        qvT_bf[64:128, qb * 128 : (qb + 1) * 128],
                        ident_bf[64:128, 64:128],
                    )
                nc.vector.tensor_copy(vp_sb[:, 4 * sc : 4 * sc + 4, 0:H], vtp[:])

            ops_by_J = {}

            def attention_blocks(J):
                """Scores run DEPTH blocks ahead of PVs in PE program order,
                so the PE never stalls waiting for the ScalarE exp. The last
                DEPTH PVs are NOT emitted here — the returned flush closure
                emits them, and the caller interleaves the next slab's
                projection chain in between to cover their exp latency."""
                ops = pout.tile([128, SC], f32)  # rows 0-63 out^T, 64 l^T
                ops_by_J[J] = ops
                njt = 4 * J + 4                  # k-tiles 0..4J+3
                pts = {}

                def score(j):
                    r = j - 4 * J                # >=0 on diagonal tiles
                    col0 = max(0, r * 128)
                    # att^T block [k=128, q]: zero-padded K^T stationary x
                    # full qvT moving (V^T rows hit the zero rows)
                    aps = pblk.tile([128, SC], f32, tag="blk")
                    nc.tensor.matmul(
                        aps[:, col0:SC],
                        lhsT=kT_pad[:, j * 128 : (j + 1) * 128],
                        rhs=qvT_bf[:, J * SC + col0 : (J + 1) * SC],
                        start=True,
                        stop=True,
                    )
                    # P^T = exp(att^T / sqrt(E)); zero the k>q corner on the
                    # diagonal block
                    pt = ptp.tile([128, SC], bf16, tag="pt")
                    nc.scalar.activation(
                        out=pt[:, col0:SC],
                        in_=aps[:, col0:SC],
                        func=Exp,
                        bias=zbias[:],
                        scale=SCALE,
                    )
                    if r >= 0:
                        nc.vector.tensor_mul(
                            pt[:, col0 : col0 + 128],
                            pt[:, col0 : col0 + 128],
                            triu[:],
                        )
                    pts[j] = (pt, col0)

                def pv(j):
                    pt, col0 = pts.pop(j)
                    # out^T[:, col0:] += V'_j^T @ P^T_j
                    nc.tensor.matmul(
                        ops[:, col0:SC],
                        lhsT=vp_sb[:, j, :],
                        rhs=pt[:, col0:SC],
                        start=(j == 0),
                        stop=(j == njt - 1),
                    )

                for j in range(njt):
                    score(j)
                    if j >= DEPTH:
                        pv(j - DEPTH)

                def flush():
                    for j in range(max(0, njt - DEPTH), njt):
                        pv(j)

                return flush

            def attention_out(J):
                # transpose back per 128-q block, divide by l, batched store
                ops = ops_by_J.pop(J)
                osb = work.tile([H + 1, SC], f32, tag="osb")
                nc.vector.tensor_copy(osb[:], ops[0 : H + 1, :])
                obs = work.tile([128, 4, H], f32, tag="ob")
                for c in range(4):
                    tp2 = psml.tile([128, H + 1], f32, tag="tr")
                    nc.tensor.transpose(
                        tp2[:],
                        osb[:, c * 128 : (c + 1) * 128],
                        ident[0 : H + 1, 0 : H + 1],
                    )
                    rcp = work.tile([128, 1], f32, tag="rcp")
                    nc.vector.reciprocal(rcp[:], tp2[:, H : H + 1])
                    nc.vector.tensor_scalar_mul(
                        obs[:, c, :], in0=tp2[:, 0:H], scalar1=rcp[:]
                    )
                # one SWDGE store per slab; keeps stores off the input rings
                nc.gpsimd.dma_start(
                    out=out[J * SC : (J + 1) * SC, :].rearrange(
                        "(c p) h -> p c h", p=128
                    ),
                    in_=obs[:],
                )

            # Schedule: the tail PVs of slab J flush BETWEEN the two
            # projection chains of slab J+1, so their exps complete under
            # chain-A matmuls; slab J's output stage runs after slab J+1's
            # projections so the PE never waits on the DVE osb copy.
            chainA(0)
            chainB(0)
            vtr(0)
            flush = attention_blocks(0)
            for sc in range(1, NSC):
                chainA(sc)
                flush()
                chainB(sc)
                vtr(sc)
                attention_out(sc - 1)
                flush = attention_blocks(sc)
            flush()
            attention_out(NSC - 1)
    return nc


def _legalize_waits(nc):
    """Split multi-wait instructions: the TPB ISA encodes one sem-wait per
    instruction and several walrus struct lowerings (Activation, self-loading
    Matmult, DMA direct2d, NoOp/Drain) reject more ("Too many sync wait
    commands"). Move excess waits onto inserted same-engine NoOps, one wait
    each. EventSemaphore handles wait lists natively - leave it."""
    from concourse import mybir

    skip = (mybir.InstEventSemaphore,)
    n = 0
    for f in nc.m.functions:
        for bb in f.blocks:
            new = []
            for inst in bb.instructions:
                si = inst.sync_info
                waits = list(si.on_wait) if si is not None else []
                if len(waits) > 1 and not isinstance(inst, skip):
                    for w in waits[:-1]:
                        n += 1
                        nop = mybir.InstNoOp(
                            name=f"I-waitsplit-{n}", ins=[], outs=[]
                        )
                        nop.engine = inst.engine
                        nop.sync_info = mybir.SyncInfo(on_wait=[w], on_update=[])
                        new.append(nop)
                    inst.sync_info = mybir.SyncInfo(
                        on_wait=[waits[-1]], on_update=list(si.on_update)
                    )
                new.append(inst)
            bb.instructions[:] = new
    return n


def _get_nc():
    if "nc" not in _CACHE:
        nc = _build_bass()
        _legalize_waits(nc)
        _CACHE["nc"] = nc
    return _CACHE["nc"]


def _bf16():
    import ml_dtypes

    return ml_dtypes.bfloat16


def _prep_x(xb):
    """[S, E] batch element -> xs[sc, p, t, s] bf16 slab-contiguous layout."""
    return np.ascontiguousarray(
        xb.T.astype(_bf16()).reshape(NET, 128, NSC, SC).transpose(2, 1, 0, 3)
    )


def _consts():
    bf16 = _bf16()
    ident = np.eye(128, dtype=np.float32)
    triu = np.triu(np.ones((128, 128), dtype=np.float32))  # 1 where k <= q
    cb = np.ascontiguousarray(
        np.concatenate([ident, triu], axis=1).astype(bf16)
    )
    cf = np.ascontiguousarray(ident)
    return cb, cf


def _in_maps(x, Wq, Wk, Wv):
    bf16 = _bf16()
    x = np.asarray(x, dtype=np.float32)
    wa = np.ascontiguousarray(
        np.concatenate(
            [np.asarray(Wq, np.float32), np.asarray(Wv, np.float32)], axis=1
        ).astype(bf16)
    )
    wb = np.ascontiguousarray(np.asarray(Wk, np.float32).astype(bf16))
    cb, cf = _consts()
    return [
        {"xs": _prep_x(x[b]), "wa": wa, "wb": wb, "cb": cb, "cf": cf}
        for b in range(B)
    ]


def kernel(x, Wq, Wk, Wv):
    from concourse.bass_utils import run_bass_kernel_spmd

    nc = _get_nc()
    in_maps = _in_maps(x, Wq, Wk, Wv)
    res = run_bass_kernel_spmd(nc, in_maps, core_ids=list(range(B)))
    return np.stack([res.results[b]["out"] for b in range(B)], axis=0)
